# revision 1
# baseline (speedup 1.0000x reference)
"""Trainium2 Bass kernel for nn_AblationAttention (sparse_attention).

Sharding: head-parallel attention (4 heads/core; batch b = core//4) with a
row-parallel output projection (each core computes a full-shape partial that
the host sums per batch).  One tiny AllReduce(max) of [vmax, -vmin] per
4-core batch group provides the global per-batch min/max for v scaling.

Self-contained: hardcodes B=2, S=2048, E=1024, H=16, D=64, 8 cores.
"""
import numpy as np

import concourse.bass as bass
import concourse.mybir as mybir
import concourse.tile as tile
from concourse import bacc
from concourse.masks import make_identity
from concourse.bass_utils import run_bass_kernel_spmd

B, S, E, H, D = 2, 2048, 1024, 16, 64
NCORES = 8
HPC = H // 4                  # 4 heads per core
NJ = S // 128                 # 16 j-blocks
NT = S // 512                 # 4 i-tiles of 512
GROUPS = [[0, 1, 2, 3], [4, 5, 6, 7]]

FAST = True                   # float32r matmuls for the big GEMMs
F32 = mybir.dt.float32
MM = mybir.dt.float32r if FAST else F32
AL = mybir.AluOpType
AX = mybir.AxisListType
AF = mybir.ActivationFunctionType


def _f32r(ap):
    return ap.bitcast(MM) if FAST else ap


def build():
    nc = bacc.Bacc("TRN2", target_bir_lowering=False, debug=False,
                   num_devices=NCORES)
    # ---- I/O ----
    xT = nc.dram_tensor("xT", [E, S], F32, kind="ExternalInput")
    wvT = nc.dram_tensor("wvT", [E, HPC * D], F32, kind="ExternalInput")
    wvb2 = nc.dram_tensor("wvb2", [2, 128], F32, kind="ExternalInput")
    wor = nc.dram_tensor("wor", [HPC * D, E], F32, kind="ExternalInput")
    wob4 = nc.dram_tensor("wob4", [1, E], F32, kind="ExternalInput")
    rcnt = nc.dram_tensor("rcnt", [1, S], F32, kind="ExternalInput")
    maskv = nc.dram_tensor("maskv", [4, 128, 512], F32, kind="ExternalInput")
    selcol = nc.dram_tensor("selcol", [128, 2], F32, kind="ExternalInput")
    konst = nc.dram_tensor("konst", [2, S], F32, kind="ExternalInput")
    sel2 = nc.dram_tensor("sel2", [2, 128], F32, kind="ExternalInput")
    out_part = nc.dram_tensor("out_part", [S, E], F32, kind="ExternalOutput")
    # collective bounce buffers
    ar_in = nc.dram_tensor("ar_in", [1, 2], F32)
    ar_out = nc.dram_tensor("ar_out", [1, 2], F32)

    with tile.TileContext(nc) as tc:
        with tc.tile_pool(name="const", bufs=1) as constp, \
             tc.tile_pool(name="persist", bufs=1) as persist:
            ident = constp.tile([128, 128], F32)
            make_identity(nc, ident)
            rcnt_bc = constp.tile([128, S], F32)
            rc_src = rcnt.ap()
            nc.sync.dma_start(out=rcnt_bc, in_=bass.AP(
                tensor=rc_src.tensor, offset=rc_src.offset,
                ap=[[0, 128]] + list(rc_src.ap[1:])))
            wob_bc = constp.tile([128, E], F32)
            wb_src = wob4.ap()
            nc.sync.dma_start(out=wob_bc, in_=bass.AP(
                tensor=wb_src.tensor, offset=wb_src.offset,
                ap=[[0, 128]] + list(wb_src.ap[1:])))
            selcol_sb = constp.tile([128, 2], MM)
            nc.sync.dma_start(out=selcol_sb, in_=_f32r(selcol.ap()))
            sel2_sb = constp.tile([2, 128], MM)
            nc.sync.dma_start(out=sel2_sb, in_=_f32r(sel2.ap()))
            wvb_sb = constp.tile([128, 2], F32)
            for m in range(2):
                nc.sync.dma_start(out=wvb_sb[:, m:m + 1], in_=wvb2.ap()[m:m + 1, :])
            mmg = constp.tile([128, 2], F32)        # [gmax, -gmin] broadcast
            a_sb = constp.tile([128, 1], F32)
            c_sb = constp.tile([128, 1], F32)
            ch_sb = constp.tile([128, 1], F32)

            # persistent activation tensors
            vsT = persist.tile([128, 2, S], MM)           # v_scaled^T (2 pairs)
            v_s = [persist.tile([128, HPC, D + 2], MM, name=f"v_s{J}")
                   for J in range(NJ)]
            gfz = [persist.tile([128, 2, S], MM, name=f"gfz{p}")
                   for p in range(2)]
            ctxT = persist.tile([128, 2, S], MM)          # context^T
            mask_sb = persist.tile([128, 4, 512], MM)
            for o in range(4):
                nc.sync.dma_start(out=mask_sb[:, o, :],
                                  in_=_f32r(maskv.ap()[o, :, :]))
            wor_sb = persist.tile([128, 2, E], MM)
            for kt in range(2):
                nc.sync.dma_start(out=wor_sb[:, kt, :],
                                  in_=_f32r(wor.ap()[128 * kt:128 * (kt + 1), :]))
            z1 = constp.tile([128, 1], F32)
            nc.vector.memset(z1, 0.0)
            onez = constp.tile([128, 2], F32)   # col0=1.0, col1=0.0
            nc.vector.memset(onez[:, 0:1], 1.0)
            nc.vector.memset(onez[:, 1:2], 0.0)
            ones_row = constp.tile([1, D], MM)
            zdummy = bass.AP(tensor=z1.tensor, offset=z1.offset,
                             ap=[list(z1.ap[0]), [0, S]])
            z1row = bass.AP(tensor=z1.tensor, offset=z1.offset,
                            ap=[[list(z1.ap[0])[0], 1], [0, D]])
            nc.vector.tensor_scalar(out=ones_row, in0=z1row, scalar1=0.0,
                                    scalar2=1.0, op0=AL.mult, op1=AL.add)

            # ---------- Phase A: v projection ----------
            with tc.tile_pool(name="phiP", bufs=1) as phiP:
                phi_raw = [phiP.tile([128, S], F32, name=f"phi{p}")
                           for p in range(2)]
                rs_sb = [phiP.tile([2, S], F32, name=f"rs{p}") for p in range(2)]

                with tc.tile_pool(name="xw", bufs=1) as xw, \
                     tc.tile_pool(name="xk", bufs=2) as xkp:
                    wvT_sb = xw.tile([128, 8, HPC * D], MM)
                    for k in range(8):
                        nc.sync.dma_start(out=wvT_sb[:, k, :],
                                          in_=_f32r(xT_slice(wvT, k)))
                    vT_sb = xw.tile([128, 2, S], F32)
                    with tc.tile_pool(name="psA", bufs=1, space="PSUM") as psA:
                        pv = [psA.tile([128, 512], F32, name=f"pv{i}")
                              for i in range(8)]
                        for k in range(8):
                            xk = xkp.tile([128, S], MM, name="xk")
                            nc.sync.dma_start(out=xk, in_=_f32r(xT_slice(xT, k)))
                            for m in range(2):
                                for t in range(NT):
                                    nc.tensor.matmul(
                                        pv[m * NT + t],
                                        wvT_sb[:, k, 128 * m:128 * (m + 1)],
                                        xk[:, 512 * t:512 * (t + 1)],
                                        start=(k == 0), stop=(k == 7))
                        for m in range(2):
                            for t in range(NT):
                                nc.vector.tensor_scalar_add(
                                    out=vT_sb[:, m, 512 * t:512 * (t + 1)],
                                    in0=pv[m * NT + t], scalar1=wvb_sb[:, m:m + 1])

                    psM_cm = tc.tile_pool(name="psM", bufs=2, space="PSUM")
                    psM = psM_cm.__enter__()
                    # ---------- min/max + AllReduce ----------
                    stat = xw.tile([128, 2], F32)
                    nc.vector.tensor_reduce(out=stat[:, 0:1], in_=vT_sb[:, :, :],
                                            op=AL.max, axis=AX.XY)
                    nc.vector.tensor_reduce(out=stat[:, 1:2], in_=vT_sb[:, :, :],
                                            op=AL.min, axis=AX.XY)
                    nc.vector.tensor_scalar_mul(out=stat[:, 1:2],
                                                in0=stat[:, 1:2], scalar1=-1.0)
                    pstat = psM.tile([2, 128], F32, name="pstat")
                    nc.tensor.transpose(pstat, stat, ident)
                    st2 = xw.tile([2, 1], F32)
                    nc.vector.tensor_reduce(out=st2, in_=pstat, op=AL.max,
                                            axis=AX.X)
                    nc.sync.dma_start(out=ar_in.ap(), in_=st2)
                    nc.gpsimd.collective_compute(
                        "AllReduce", AL.max, replica_groups=GROUPS,
                        ins=[ar_in.ap()], outs=[ar_out.ap()])
                    aro = ar_out.ap()
                    nc.sync.dma_start(out=mmg, in_=bass.AP(
                        tensor=aro.tensor, offset=aro.offset,
                        ap=[[0, 128]] + list(aro.ap[1:])))
                    # a = 1/(gmax - gmin + 1e-8); c = -gmin * a
                    nc.vector.tensor_tensor(out=a_sb, in0=mmg[:, 0:1],
                                            in1=mmg[:, 1:2], op=AL.add)
                    nc.vector.tensor_scalar_add(out=a_sb, in0=a_sb, scalar1=1e-8)
                    nc.vector.reciprocal(out=a_sb, in_=a_sb)
                    nc.vector.tensor_tensor(out=c_sb, in0=mmg[:, 1:2],
                                            in1=a_sb, op=AL.mult)
                    nc.vector.tensor_scalar_add(out=ch_sb, in0=c_sb, scalar1=0.5)

                    # ---------- scan (pre-AR) + vsT + v_s ----------
                    for p in range(2):
                        nc.vector.tensor_tensor_scan(
                            out=phi_raw[p], data0=vT_sb[:, p, :], data1=zdummy,
                            initial=0.0, op0=AL.add, op1=AL.add)
                        nc.vector.tensor_tensor(out=phi_raw[p], in0=phi_raw[p],
                                                in1=rcnt_bc, op=AL.mult)
                    nc.vector.tensor_scalar(
                        out=vsT[:, :, :], in0=vT_sb[:, :, :], scalar1=a_sb,
                        scalar2=c_sb, op0=AL.mult, op1=AL.add)
                    for J in range(NJ):
                        ptr = psM.tile([128, 256], F32, name="ptr")
                        for m in range(2):
                            nc.tensor.transpose(
                                ptr[:, 128 * m:128 * (m + 1)],
                                vT_sb[:, m, 128 * J:128 * (J + 1)], ident)
                        nc.scalar.activation(
                            out=v_s[J][:, :, 0:D],
                            in_=ptr.rearrange("p (h d) -> p h d", h=HPC),
                            func=AF.Copy)
                        nc.vector.tensor_copy(
                            out=v_s[J][:, :, D:D + 2],
                            in_=bass.AP(tensor=onez.tensor, offset=onez.offset,
                                        ap=[list(onez.ap[0]), [0, HPC], [1, 2]]))
                    psM_cm.__exit__(None, None, None)

                # ---------- Phase C: phi -> gene fitness ----------
                with tc.tile_pool(name="cw", bufs=2) as cw, \
                     tc.tile_pool(name="psS", bufs=2, space="PSUM") as psS, \
                     tc.tile_pool(name="psB", bufs=2, space="PSUM") as psB:
                    for p in range(2):
                        denom = cw.tile([128, S], F32, name="denom")
                        nc.vector.tensor_scalar(
                            out=denom, in0=phi_raw[p], scalar1=a_sb,
                            scalar2=ch_sb, op0=AL.mult, op1=AL.add)
                        rd = cw.tile([128, S], F32, name="rd")
                        nc.vector.reciprocal_approx_fast(out=rd, in_=denom)
                        rdr = cw.tile([128, S], MM, name="rdr")
                        nc.gpsimd.tensor_copy(out=rdr, in_=rd)
                        rsr = cw.tile([2, S], MM, name="rsr")
                        nc.gpsimd.tensor_scalar(
                            out=gfz[p][64:128, 0, :], in0=rd[64:128, :],
                            scalar1=0.0, scalar2=0.0, op0=AL.mult, op1=AL.add)
                        nc.gpsimd.tensor_scalar(
                            out=gfz[p][0:64, 1, :], in0=rd[0:64, :],
                            scalar1=0.0, scalar2=0.0, op0=AL.mult, op1=AL.add)
                        for t in range(NT):
                            sl = slice(512 * t, 512 * (t + 1))
                            si = psS.tile([2, 512], F32, name="si")
                            nc.tensor.matmul(si, selcol_sb, rdr[:, sl],
                                             start=True, stop=True)
                            nc.vector.reciprocal_approx_fast(out=rs_sb[p][:, sl],
                                                             in_=si)
                            nc.vector.tensor_copy(out=rsr[:, sl],
                                                  in_=rs_sb[p][:, sl])
                            bc = psB.tile([128, 512], F32, name="bc")
                            nc.tensor.matmul(bc, sel2_sb, rsr[:, sl],
                                             start=True, stop=True)
                            nc.vector.tensor_tensor(
                                out=gfz[p][0:64, 0, sl], in0=bc[0:64, :],
                                in1=rd[0:64, sl], op=AL.mult)
                            nc.vector.tensor_tensor(
                                out=gfz[p][64:128, 1, sl], in0=bc[64:128, :],
                                in1=rd[64:128, sl], op=AL.mult)

            # ---------- Phase D: attention ----------
            with tc.tile_pool(name="et", bufs=5) as etp, \
                 tc.tile_pool(name="zw", bufs=8) as zw:
                psD_cm = [tc.tile_pool(name="psO", bufs=3, space="PSUM"),
                          tc.tile_pool(name="psZ", bufs=2, space="PSUM"),
                          tc.tile_pool(name="psAV", bufs=3, space="PSUM")]
                psO, psZ, psAV = [cm.__enter__() for cm in psD_cm]
                for p in range(2):
                    for s in range(2):
                        lh = 2 * p + s
                        for T in range(NT):
                            pav = psAV.tile([D + 2, 512], F32, name="pav")
                            for J in range(4 * T + 4):
                                off = J - 4 * T
                                c0 = 128 * off if off > 0 else 0
                                po = psO.tile([128, 512], F32, name="po")
                                nc.tensor.matmul(
                                    po[:, c0:512],
                                    vsT[:, p, 128 * J:128 * (J + 1)],
                                    gfz[p][:, s, 512 * T + c0:512 * (T + 1)],
                                    start=True, stop=True)
                                et = etp.tile([128, 512], MM, name="et")
                                if c0 > 0:
                                    nc.vector.tensor_scalar(
                                        out=et[:, 0:c0],
                                        in0=mask_sb[:, 0, 0:c0], scalar1=0.0,
                                        scalar2=0.0, op0=AL.mult, op1=AL.add)
                                nc.scalar.activation(out=et[:, c0:512],
                                                     in_=po[:, c0:512],
                                                     func=AF.Exp)
                                if off >= 0:
                                    nc.gpsimd.tensor_tensor(
                                        out=et[:, c0:512], in0=et[:, c0:512],
                                        in1=mask_sb[:, off, c0:512], op=AL.mult)
                                nc.tensor.matmul(
                                    pav, v_s[J][:, lh, :], et,
                                    start=(J == 0), stop=(J == 4 * T + 3))
                            # epilogue: normalize by Z (row 64 of pav)
                            o2t = etp.tile([D, 512], F32, name="o2t")
                            nc.scalar.activation(out=o2t, in_=pav[0:D, :],
                                                 func=AF.Copy)
                            zr = zw.tile([1, 512], F32, name="zr")
                            nc.vector.tensor_copy(out=zr, in_=pav[D:D + 1, :])
                            rz = zw.tile([1, 512], F32, name="rz")
                            nc.vector.reciprocal_approx_fast(out=rz, in_=zr)
                            rzr = zw.tile([1, 512], MM, name="rzr")
                            nc.vector.tensor_copy(out=rzr, in_=rz)
                            bcz = psZ.tile([D, 512], F32, name="bcz")
                            nc.tensor.matmul(bcz, ones_row, rzr,
                                             start=True, stop=True)
                            nc.vector.tensor_tensor(
                                out=ctxT[64 * (lh % 2):64 * (lh % 2) + D,
                                         lh // 2, 512 * T:512 * (T + 1)],
                                in0=o2t, in1=bcz, op=AL.mult)

                for cm in reversed(psD_cm):
                    cm.__exit__(None, None, None)

                # ---------- Phase F: output projection (partial) ----------
                with tc.tile_pool(name="osb", bufs=3) as osbp, \
                     tc.tile_pool(name="psF", bufs=2, space="PSUM") as psF:
                    for mt in range(NJ):
                        osb = osbp.tile([128, E], F32, name="osb")
                        for nt in range(2):
                            poo = psF.tile([128, 512], F32, name="poo")
                            for kt in range(2):
                                nc.tensor.matmul(
                                    poo,
                                    ctxT[:, kt, 128 * mt:128 * (mt + 1)],
                                    wor_sb[:, kt, 512 * nt:512 * (nt + 1)],
                                    start=(kt == 0), stop=(kt == 1))
                            nc.vector.tensor_tensor(
                                out=osb[:, 512 * nt:512 * (nt + 1)], in0=poo,
                                in1=wob_bc[:, 512 * nt:512 * (nt + 1)], op=AL.add)
                        nc.sync.dma_start(
                            out=out_part.ap()[128 * mt:128 * (mt + 1), :], in_=osb)
    nc.compile()
    return nc


def xT_slice(t, k):
    return t.ap()[128 * k:128 * (k + 1), :]


def make_host_inputs(x, wv_w, wv_b, wo_w, wo_b):
    """Per-core input dicts (host-side sharding)."""
    rcnt = (1.0 / (np.arange(S, dtype=np.float64) + 1.0)).astype(np.float32)
    maskv = np.zeros((4, 128, 512), np.float32)
    jj = np.arange(128)[:, None]
    ii = np.arange(512)[None, :]
    for o in range(4):
        maskv[o] = (128 * o + jj <= ii)
    selcol = np.zeros((128, 2), np.float32)
    selcol[:64, 0] = 1.0
    selcol[64:, 1] = 1.0
    sel2 = np.ascontiguousarray(selcol.T)
    woT = np.ascontiguousarray(wo_w.T)
    in_maps = []
    for c in range(NCORES):
        b, q = c // 4, c % 4
        csl = slice(HPC * D * q, HPC * D * (q + 1))
        in_maps.append({
            "xT": np.ascontiguousarray(x[b].T),
            "wvT": np.ascontiguousarray(wv_w[csl, :].T),
            "wvb2": np.ascontiguousarray(wv_b[csl].reshape(2, 128)),
            "wor": np.ascontiguousarray(woT[csl, :]),
            "wob4": np.ascontiguousarray((wo_b / 4.0).reshape(1, E)),
            "rcnt": rcnt.reshape(1, S),
            "maskv": maskv,
            "selcol": selcol,
            "konst": np.stack([np.zeros(S, np.float32), np.ones(S, np.float32)]),
            "sel2": sel2,
        })
    return in_maps


_NC_CACHE = {}


def _get_nc():
    if "nc" not in _NC_CACHE:
        _NC_CACHE["nc"] = build()
    return _NC_CACHE["nc"]


def _assemble(results):
    out = np.zeros((B, S, E), np.float32)
    for c in range(NCORES):
        out[c // 4] += results[c]["out_part"]
    return out


def kernel(x, wv_w, wv_b, wo_w, wo_b):
    x = np.asarray(x, np.float32)
    in_maps = make_host_inputs(
        x, np.asarray(wv_w, np.float32), np.asarray(wv_b, np.float32),
        np.asarray(wo_w, np.float32), np.asarray(wo_b, np.float32))
    nc = _get_nc()
    res = run_bass_kernel_spmd(nc, in_maps, core_ids=list(range(NCORES)))
    return _assemble(res.results)


def run_traced(x, wv_w, wv_b, wo_w, wo_b, trace_cores=None):
    in_maps = make_host_inputs(
        np.asarray(x, np.float32), np.asarray(wv_w, np.float32),
        np.asarray(wv_b, np.float32), np.asarray(wo_w, np.float32),
        np.asarray(wo_b, np.float32))
    nc = _get_nc()
    res = run_bass_kernel_spmd(nc, in_maps, core_ids=list(range(NCORES)),
                               trace=True, trace_cores=trace_cores)
    return _assemble(res.results), res

